# revision 39
# baseline (speedup 1.0000x reference)
"""Bass/Trainium2 kernel for nn_EquivariantReynoldsWrap.

The reference module is linear in x: for every pixel,
    out = (1/G) * sum_g BlockDiag(A_g) @ Wf @ BlockDiag(Ainv_g) @ x_pixel
so the whole pipeline collapses into one 64x64 channel-mixing matrix M,
computed on host (cheap). The device work is a single 1x1-conv matmul
out[b] = M @ x[b] with x[b] viewed as (64, H*W).

Sharding: data-parallel over B across the 8 cores (1 batch each).
Per core the two halves of the pixel axis are interleaved on the
partition axis (partition p = channel p//2, half p%2) and the stationary
weight is the 128x128 interleaved block-diagonal of M^T, so each
512-column matmul covers 1024 pixels.

Raw bacc (no TileContext): hand-rolled semaphores, minimal head/tail.
"""

import numpy as np

import concourse.bacc as bacc
import concourse.bass as bass
from concourse import mybir
from concourse.bass_utils import run_bass_kernel_spmd

B, C, H, W_SP = 8, 64, 64, 64
COUT = 64
HW = H * W_SP          # 4096 pixels per batch
HALF = HW // 2         # 2048 -> stacked column count per core
N_CORES = 8

CH = 512               # columns per pipeline chunk
N_CHUNKS = HALF // CH  # 4
N_WARM = 7             # bf16 warm-up matmuls (HAM un-throttle)
USE_F32R = False       # single-pass matmuls, ~1.5e-4 rel err (vs 1e-7 fp32)

TRACE = False          # test.py flips this to profile
_cached_nc = None


def _build_nc():
    global _cached_nc
    if _cached_nc is not None:
        return _cached_nc

    in_dt = mybir.dt.float32r if USE_F32R else mybir.dt.float32
    f32 = mybir.dt.float32

    nc = bacc.Bacc(
        "TRN2",
        target_bir_lowering=False,
        debug=False,
        enable_asserts=False,
        num_devices=N_CORES,
    )
    xd = nc.dram_tensor("x", [C, HW], in_dt, kind="ExternalInput").ap()
    wd = nc.dram_tensor("w", [128, 128], in_dt, kind="ExternalInput").ap()
    yd = nc.dram_tensor("y", [COUT, HW], f32, kind="ExternalOutput").ap()

    # [64, 2, t] c-major outer dims: the DMA pairs partition p with
    # (c=p//2, s=p%2); the outer dim of 64 spreads each transfer across
    # all 16 SDMA engines (an outer dim of 2 used only 2 of them).
    xr = xd.rearrange("c (s t) -> c s t", s=2)
    yr = yd.rearrange("c (s t) -> c s t", s=2)

    with (
        nc.sbuf_tensor("wt", [128, 128], in_dt) as wt_t,
        nc.sbuf_tensor("xt", [128, HALF], in_dt) as xt_t,
        nc.sbuf_tensor("ot", [128, HALF], f32) as ot_t,
        nc.sbuf_tensor("zt", [128, 512], mybir.dt.bfloat16) as zt_t,
        nc.psum_tensor([128, HALF], f32) as ps_t,
        nc.psum_tensor([128, 512], f32) as wps_t,
        nc.semaphore("s_w") as s_w,      # weights DMA done
        # one sem per x-chunk DMA: a sem shared by two DMAs on one ring
        # reaches 16 from a MIX of the two transfers' per-engine incs
        nc.semaphore("s_x0") as s_x0,
        nc.semaphore("s_x1") as s_x1,
        nc.semaphore("s_x2") as s_x2,
        nc.semaphore("s_x3") as s_x3,
        nc.semaphore("s_z") as s_z,      # warmup tile zeroed
        nc.semaphore("s_mm") as s_mm,    # matmul per chunk
        nc.semaphore("s_cpv") as s_cpv,  # DVE copies (chunks 0, 2)
        nc.semaphore("s_cpa") as s_cpa,  # ACT copies (chunks 1, 3)
        nc.semaphore("s_y") as s_y,      # out DMAs
    ):
        wt = wt_t.ap()
        xt = xt_t.ap()
        ot = ot_t.ap()
        zt = zt_t.ap()
        ps = ps_t.ap()
        wps = wps_t.ap()

        def cs(i):
            return slice(i * CH, (i + 1) * CH)

        # Linear emission into the entry basic block (no nc.Block): avoids
        # the per-engine body branches (I$ misses) and the Block exit
        # barrier; the walrus-generated NEFF epilogue handles quiescence
        # and zeroes all semaphores for re-execution.
        sync, scalar, tensor, vector, gpsimd = (
            nc.sync, nc.scalar, nc.tensor, nc.vector, nc.gpsimd
        )

        gpsimd.memset(zt[:], 0.0).then_inc(s_z)

        # ring assignment by completion order: each ring's DMAs complete
        # serially, so chunk i's gate is matched to the i-th completing
        # transfer across the two rings:
        #   scalar 1st -> c0, sync 2nd (after small w) -> c1,
        #   sync 3rd -> c2, scalar 2nd -> c3
        sync.dma_start(wt[:], wd[:]).then_inc(s_w, 16)
        sync.dma_start(xt[:, cs(1)], xr[:, :, cs(1)]).then_inc(s_x1, 16)
        sync.dma_start(xt[:, cs(2)], xr[:, :, cs(2)]).then_inc(s_x2, 16)
        scalar.dma_start(xt[:, cs(0)], xr[:, :, cs(0)]).then_inc(s_x0, 16)
        scalar.dma_start(xt[:, cs(3)], xr[:, :, cs(3)]).then_inc(s_x3, 16)

        # HAM warm-up on zeroed bf16 tile (1 HW pass each)
        tensor.wait_ge(s_z, 1)
        for _ in range(N_WARM):
            tensor.matmul(wps[:], zt[:, :128], zt[:])

        # A matmul's sem update fires at instruction retire (last column
        # ENTERS the array); the ~128-cycle systolic drain is still
        # writing PSUM then. Chunk i's drain is covered by chunk i+1's
        # matmul (s_mm >= i+2); only the last chunk needs an explicit
        # short guard matmul to carry its inc.
        tensor.wait_ge(s_w, 16)
        tensor.wait_ge(s_x0, 16)
        tensor.matmul(ps[:, cs(0)], wt[:], xt[:, cs(0)]).then_inc(s_mm)
        tensor.wait_ge(s_x1, 16)
        tensor.matmul(ps[:, cs(1)], wt[:], xt[:, cs(1)]).then_inc(s_mm)
        tensor.wait_ge(s_x2, 16)
        tensor.matmul(ps[:, cs(2)], wt[:], xt[:, cs(2)]).then_inc(s_mm)
        tensor.wait_ge(s_x3, 16)
        tensor.matmul(ps[:, cs(3)], wt[:], xt[:, cs(3)]).then_inc(s_mm)
        tensor.matmul(wps[:, :128], zt[:, :128], zt[:, :128]).then_inc(s_mm)

        # copies: DVE takes chunks 0, 2; ACT takes 1, 3. The out-DMA
        # triggers are sequencer-class ops, so each gates on the copy's
        # completion sem (queue order alone does NOT order it after the
        # datapath).
        vector.wait_ge(s_mm, 2)
        vector.tensor_copy(ot[:, cs(0)], ps[:, cs(0)]).then_inc(s_cpv)
        vector.wait_ge(s_mm, 4)
        vector.tensor_copy(ot[:, cs(2)], ps[:, cs(2)]).then_inc(s_cpv)

        scalar.wait_ge(s_mm, 3)
        scalar.copy(ot[:, cs(1)], ps[:, cs(1)]).then_inc(s_cpa)
        scalar.wait_ge(s_cpa, 1)
        scalar.dma_start(yr[:, :, cs(1)], ot[:, cs(1)]).then_inc(s_y, 16)
        scalar.wait_ge(s_mm, 5)
        scalar.copy(ot[:, cs(3)], ps[:, cs(3)]).then_inc(s_cpa)
        scalar.wait_ge(s_cpa, 2)
        scalar.dma_start(yr[:, :, cs(3)], ot[:, cs(3)]).then_inc(s_y, 16)

        sync.wait_ge(s_cpv, 1)
        sync.dma_start(yr[:, :, cs(0)], ot[:, cs(0)]).then_inc(s_y, 16)
        sync.wait_ge(s_cpv, 2)
        sync.dma_start(yr[:, :, cs(2)], ot[:, cs(2)]).then_inc(s_y, 16)
        # hold the program open until every output write landed (running
        # the NEFF epilogue concurrently with in-flight output DMAs was
        # observed to wedge the device)
        sync.wait_ge(s_y, 64)

    nc.compile()
    _cached_nc = nc
    return nc


def _fuse_weights(group_tensor, group_tensor_inv, Wf):
    A = np.asarray(group_tensor, np.float64)
    Ai = np.asarray(group_tensor_inv, np.float64)
    Wf64 = np.asarray(Wf, np.float64)
    G, CG, _ = A.shape
    n = C // CG
    eye = np.eye(n)
    M = np.zeros((COUT, C))
    for g in range(G):
        M += np.kron(eye, A[g]) @ Wf64 @ np.kron(eye, Ai[g])
    M /= G
    MT = np.ascontiguousarray(M.T).astype(np.float32)
    # interleaved packing: x-tile partition p holds channel p//2 of pixel
    # half p%2; out partition q holds channel q//2 of half q%2.
    W2T = np.zeros((128, 128), np.float32)
    W2T[0::2, 0::2] = MT
    W2T[1::2, 1::2] = MT
    return W2T


def kernel(x, group_tensor, group_tensor_inv, Wf):
    nc = _build_nc()
    W2T = _fuse_weights(group_tensor, group_tensor_inv, Wf)
    x = np.ascontiguousarray(np.asarray(x, np.float32))

    in_maps = [
        {"x": x[b].reshape(C, HW), "w": W2T} for b in range(B)
    ]
    res = run_bass_kernel_spmd(
        nc, in_maps, core_ids=list(range(N_CORES)), trace=TRACE
    )
    if TRACE:
        kernel.last_results = res
    y = np.stack(
        [res.results[b]["y"].reshape(COUT, H, W_SP) for b in range(B)]
    )
    return y
